# revision 1
# baseline (speedup 1.0000x reference)
"""Self-contained TRN2 Bass kernel for axial attention (nn_AxialAttention).

kernel(**inputs) takes FULL inputs (x [8,128,128,512], Wq/Wk/Wv/Wo [512,512],
bo [512]) and returns the FULL output [8,128,128,512] (float32).

Strategy: data-parallel over N across 8 NeuronCores (core c computes image c).
Per core: fp32r projections/output matmul, bf16 attention middle, softmax
without max-subtraction (logits ~N(0,1)), PE transposes for X^T and A^T,
engine-balanced copies (ACT=exp only, DVE=psum exits, per-head accum sums).
"""
import sys
sys.path.insert(0, "/opt/trn_rl_repo")
sys.path.insert(0, "/root/.axon_site/_ro/trn_rl_repo")

"""Axial attention Bass kernel for TRN2 — builder shared by test.py and kernel.py.

Problem: x [N=8, H=128, W=128, C=512], attention along H (8 heads, head dim 64):
  per (n, w): seq = x[n, :, w, :] [128, 512]
  q/k/v = seq @ W{q,k,v}.T ; per head S = q k^T/8 ; A = softmax_j(S) ; out = A v
  O = out @ Wo.T + bo  -> out[n, :, w, :]

Sharding: data-parallel over N — core c computes image n=c entirely.

Per-core layout strategy (tokens t = h, seqs s = w, 4 seqs per block):
  X_blk  [128 t, 4 s, 512 c]   <- DMA (f32r)
  X^T    [128 c_loc, 4 jc, 4 s, 128 t]  via PE transpose (f32r)
  Q^T/K^T [128 co_loc, 4 co, 4 s, 128 t] = W^T.T @ X^T  (f32r matmul, bf16 out)
  V      [128 t, 4 s, 512 c]  (bf16 out)
  S_g    [128 i, 128 j] psum = Q^T_g.T @ K^T_g  (bf16)
  A      exp(S/8) -> [128 i, 8 g, 128 j] bf16 + per-g rowsum (ACT accum)
  AN     A * (1/rowsum) broadcast  (DVE)
  A^T    per-head DMA xbar transpose -> [128 j, 8 g, 128 i] bf16
  outT   [128 c_loc, 4 jc, 128 i] psum = V_g.T...: lhsT=V_g, rhs=A^T_g
  O      [128 t, 512 co] = outT.T @ Wo^T (f32r) + bo
"""
import numpy as np

import concourse.bass as bass
import concourse.bacc as bacc
import concourse.tile as tile
from concourse import mybir

F32 = mybir.dt.float32
F32R = mybir.dt.float32r
BF16 = mybir.dt.bfloat16
EXP = mybir.ActivationFunctionType.Exp

H = 128   # tokens per sequence (attention axis)
W = 128   # sequences per core
C = 512
G = 8     # heads
GP = C // G  # 64
BLK = 4   # sequences per block
NBLK = W // BLK
NCHUNK = C // 128  # 4 k-chunks


def build_kernel(num_cores=8, attn_f32=False, w_total=W, reps=1, an_engine='dve', at_engine='dve', psum_bufs=(3,2,3), exp_mode='perhead', attn_mode='std', sbufs=None, psum_unified=False):
    """Build + compile the Bass module. Returns nc.

    reps>1 wraps the whole computation in a dynamic loop (for timing by
    wall-clock differencing; results are identical, just recomputed)."""
    nblk = w_total // BLK
    nc = bacc.Bacc("TRN2", target_bir_lowering=False, debug=False,
                   num_devices=num_cores)

    x_d = nc.dram_tensor("x", [H, w_total, C], F32R, kind="ExternalInput").ap()
    wq_d = nc.dram_tensor("wqT", [C, C], F32R, kind="ExternalInput").ap()
    wk_d = nc.dram_tensor("wkT", [C, C], F32R, kind="ExternalInput").ap()
    wv_d = nc.dram_tensor("wvT", [C, C], F32R, kind="ExternalInput").ap()
    wo_d = nc.dram_tensor("woT", [C, C], F32R, kind="ExternalInput").ap()
    bo_d = nc.dram_tensor("bo", [C], F32, kind="ExternalInput").ap()
    id_d = nc.dram_tensor("ident", [128, 128], F32R, kind="ExternalInput").ap()
    out_d = nc.dram_tensor("out", [H, w_total, C], F32, kind="ExternalOutput").ap()

    AMID = F32R if attn_f32 else BF16  # dtype of attention middle section

    sb_bufs = sbufs or {}
    def B(name, d):
        return sb_bufs.get(name, d)
    with tile.TileContext(nc) as tc:
        with tc.tile_pool(name="consts", bufs=1) as consts, \
             tc.tile_pool(name="px", bufs=B('px', 2)) as px, \
             tc.tile_pool(name="pxt", bufs=B('pxt', 2)) as pxt, \
             tc.tile_pool(name="pqt", bufs=B('pqt', 2)) as pqt, \
             tc.tile_pool(name="pv", bufs=B('pv', 2)) as pv, \
             tc.tile_pool(name="pa", bufs=B('pa', 3)) as pa, \
             tc.tile_pool(name="pstat", bufs=B('pstat', 4)) as pstat, \
             tc.tile_pool(name="pot", bufs=B('pot', 2)) as pot, \
             tc.tile_pool(name="po", bufs=B('po', 2)) as po, \
             tc.tile_pool(name="pdram", bufs=6, space="DRAM") as pdram, \
             tc.tile_pool(name="psf", bufs=(8 if psum_unified else psum_bufs[0]), space="PSUM") as psf, \
             tc.tile_pool(name="pss", bufs=psum_bufs[1], space="PSUM") as _pss, \
             tc.tile_pool(name="psb", bufs=psum_bufs[2], space="PSUM") as _psb:
            if psum_unified:
                class _U:
                    _n = [0]
                    def tile(self, shape, dtype, tag=None):
                        self._n[0] += 1
                        return psf.tile(shape, dtype, tag="f",
                                        name=f"u{self._n[0]}")
                pss = psb = _U()
            else:
                pss, psb = _pss, _psb

            # ---- constants ----
            wq_sb = consts.tile([128, NCHUNK, C], F32R, tag="wq")
            wk_sb = consts.tile([128, NCHUNK, C], F32R, tag="wk")
            wv_sb = consts.tile([128, NCHUNK, C], F32R, tag="wv")
            wo_sb = consts.tile([128, NCHUNK, C], F32R, tag="wo")
            for w_sb, w_d in ((wq_sb, wq_d), (wk_sb, wk_d), (wv_sb, wv_d),
                              (wo_sb, wo_d)):
                nc.sync.dma_start(w_sb[:], w_d.rearrange("(j p) c -> p j c", p=128))
            bo_sb = consts.tile([128, C], F32, tag="bo")
            nc.sync.dma_start(
                bo_sb[:],
                bo_d.rearrange("(o c) -> o c", o=1).broadcast_to((128, C)))
            id_sb = consts.tile([128, 128], F32R, tag="id")
            nc.sync.dma_start(id_sb[:], id_d[:])
            id_bf = consts.tile([128, 128], BF16, tag="idbf")
            nc.vector.tensor_copy(id_bf[:], id_sb[:].bitcast(F32))
            ones_bf = consts.tile([128, 128], BF16, tag="ones")
            nc.vector.memset(ones_bf[:], 1.0)

            state = {}

            def front(b):
                X_blk = px.tile([128, BLK, C], F32R, tag="x")
                nc.sync.dma_start(X_blk[:], x_d[:, b * BLK:(b + 1) * BLK, :])
                XT_sb = pxt.tile([128, NCHUNK, BLK, 128], F32R, tag="xt")
                for s in range(BLK):
                    XT_ps = psf.tile([128, NCHUNK, 128], F32R, tag="f")
                    for jc in range(NCHUNK):
                        nc.tensor.transpose(
                            XT_ps[:, jc, :],
                            X_blk[:, s, jc * 128:(jc + 1) * 128], id_sb[:])
                    nc.vector.tensor_copy(XT_sb[:, :, s, :], XT_ps[:])
                QT = pqt.tile([128, NCHUNK, BLK, 128], AMID, tag="qt")
                KT = pqt.tile([128, NCHUNK, BLK, 128], AMID, tag="kt")
                for w_sb, dst in ((wq_sb, QT), (wk_sb, KT)):
                    for co in range(NCHUNK):
                        PT = psf.tile([128, BLK * 128], F32, tag="f")
                        for jc in range(NCHUNK):
                            nc.tensor.matmul(
                                PT[:],
                                lhsT=w_sb[:, jc, co * 128:(co + 1) * 128],
                                rhs=XT_sb[:, jc, :, :],
                                start=(jc == 0), stop=(jc == NCHUNK - 1))
                        nc.vector.tensor_copy(dst[:, co, :, :], PT[:])
                V = pv.tile([128, BLK, C], AMID, tag="v")
                for s in range(BLK):
                    VP = psf.tile([128, C], F32, tag="f")
                    for jc in range(NCHUNK):
                        nc.tensor.matmul(
                            VP[:], lhsT=XT_sb[:, jc, s, :],
                            rhs=wv_sb[:, jc, :],
                            start=(jc == 0), stop=(jc == NCHUNK - 1))
                    nc.vector.tensor_copy(V[:, s, :], VP[:])
                state[b] = (QT, KT, V)

            def back(b):
                QT, KT, V = state.pop(b)
                O_sb = po.tile([128, BLK, C], F32, tag="o")
                for s in range(BLK):
                    A = pa.tile([128, G, 128], AMID, tag="a")
                    sums = pstat.tile([128, G], F32, tag="sums")
                    # Even heads (PE row-group 0) and odd heads (row-group 1)
                    # run concurrently in the array -> MUST land in different
                    # PSUM banks (same-bank concurrent row-group writes hang).
                    S_e = pss.tile([128, G // 2, 128], F32, tag="s")
                    S_o = pss.tile([128, G // 2, 128], F32, tag="s")
                    for g in range(G):
                        p0 = 64 * (g % 2)
                        S_ps = S_e if g % 2 == 0 else S_o
                        nc.tensor.matmul(
                            S_ps[:, g // 2, :],
                            lhsT=QT[p0:p0 + 64, g // 2, s, :],
                            rhs=KT[p0:p0 + 64, g // 2, s, :],
                            start=True, stop=True)
                    if exp_mode == 'perhead':
                        for g in range(G):
                            S_ps = S_e if g % 2 == 0 else S_o
                            nc.scalar.activation(
                                A[:, g, :], S_ps[:, g // 2, :], EXP,
                                scale=1.0 / np.sqrt(GP),
                                accum_out=sums[:, g:g + 1])
                    else:
                        # batched: 2 exps; A slices by even/odd heads; sums via
                        # DVE reduce over [128, 4, 128] per half
                        a_ev = A[:, 0:G:2, :]
                        a_od = A[:, 1:G:2, :]
                        nc.scalar.activation(a_ev, S_e[:], EXP,
                                             scale=1.0 / np.sqrt(GP))
                        nc.scalar.activation(a_od, S_o[:], EXP,
                                             scale=1.0 / np.sqrt(GP))
                        se = sums[:, 0:G:2].rearrange("p (g o) -> p g o", o=1)
                        so = sums[:, 1:G:2].rearrange("p (g o) -> p g o", o=1)
                        nc.vector.reduce_sum(se, a_ev, axis=mybir.AxisListType.X)
                        nc.vector.reduce_sum(so, a_od, axis=mybir.AxisListType.X)
                    rcp = pstat.tile([128, G], F32, tag="rcp")
                    nc.vector.reciprocal(rcp[:], sums[:])
                    AN = pa.tile([128, G, 128], AMID, tag="an")
                    rcp_b = rcp[:].rearrange("p (o g) -> p o g", o=1) \
                        .rearrange("p o g -> p g o") \
                        .broadcast_to((128, G, 128))
                    if an_engine == 'pool':
                        nc.gpsimd.tensor_mul(AN[:], A[:], rcp_b)
                    else:
                        nc.vector.tensor_mul(AN[:], A[:], rcp_b)
                    # A^T via PE transposes (xbar DMA transpose races with
                    # concurrent DRAM DMA traffic on this HW/runtime).
                    AT = pa.tile([128, G, 128], AMID, tag="at")
                    idt = id_bf if AMID == BF16 else id_sb
                    for half in range(2):
                        ATp = psb.tile([128, G // 2, 128], AMID, tag="b")
                        for gg in range(G // 2):
                            g = half * (G // 2) + gg
                            nc.tensor.transpose(ATp[:, gg, :], AN[:, g, :],
                                                idt[:])
                        eng = nc.vector if at_engine == 'dve' else nc.scalar
                        if at_engine == 'dve':
                            nc.vector.tensor_copy(
                                AT[:, half * (G // 2):(half + 1) * (G // 2), :],
                                ATp[:])
                        else:
                            nc.scalar.copy(
                                AT[:, half * (G // 2):(half + 1) * (G // 2), :],
                                ATp[:])
                    OT_ps = psb.tile([128, NCHUNK, 128], F32, tag="b")
                    for g in range(G):
                        p0 = 64 * (g % 2)
                        nc.tensor.matmul(
                            OT_ps[p0:p0 + 64, g // 2, :],
                            lhsT=V[:, s, 64 * g:64 * (g + 1)],
                            rhs=AT[:, g, :],
                            start=True, stop=True)
                    OT_sb = pot.tile([128, NCHUNK, 128], F32R, tag="ot")
                    nc.vector.tensor_copy(OT_sb[:], OT_ps[:])
                    O_ps = psb.tile([128, C], F32, tag="b")
                    for jc in range(NCHUNK):
                        nc.tensor.matmul(
                            O_ps[:], lhsT=OT_sb[:, jc, :],
                            rhs=wo_sb[:, jc, :],
                            start=(jc == 0), stop=(jc == NCHUNK - 1))
                    nc.vector.tensor_add(O_sb[:, s, :], O_ps[:], bo_sb[:])
                nc.sync.dma_start(out_d[:, b * BLK:(b + 1) * BLK, :], O_sb[:])

            def back_st(b):
                # S^T-direct attention: exp(S^T) IS A^T (no transposes, no
                # normalization multiply on A). Softmax sums via GpSimd
                # cross-partition reduce; normalization folded into the
                # OT psum->sbuf copy using a reciprocal tile broadcast
                # through a DRAM scratch.
                QT, KT, V = state.pop(b)
                O_sb = po.tile([128, BLK, C], F32, tag="o")
                for s in range(BLK):
                    AT = pa.tile([128, G, 128], AMID, tag="at")
                    S_e = pss.tile([128, G // 2, 128], F32, tag="s")
                    S_o = pss.tile([128, G // 2, 128], F32, tag="s")
                    for g in range(G):
                        p0 = 64 * (g % 2)
                        S_ps = S_e if g % 2 == 0 else S_o
                        # lhsT=K^T, rhs=Q^T  ->  S^T[j, i]
                        nc.tensor.matmul(
                            S_ps[:, g // 2, :],
                            lhsT=KT[p0:p0 + 64, g // 2, s, :],
                            rhs=QT[p0:p0 + 64, g // 2, s, :],
                            start=True, stop=True)
                    if exp_mode == 'perhead':
                        for g in range(G):
                            S_ps = S_e if g % 2 == 0 else S_o
                            nc.scalar.activation(
                                AT[:, g, :], S_ps[:, g // 2, :], EXP,
                                scale=1.0 / np.sqrt(GP))
                    else:
                        nc.scalar.activation(AT[:, 0:G:2, :], S_e[:], EXP,
                                             scale=1.0 / np.sqrt(GP))
                        nc.scalar.activation(AT[:, 1:G:2, :], S_o[:], EXP,
                                             scale=1.0 / np.sqrt(GP))
                    # sums[g, i] = sum_j A^T[j, g, i]  (partition reduce)
                    sums = pstat.tile([1, G, 128], F32, tag="sums")
                    nc.gpsimd.reduce_sum(sums[:], AT[:],
                                         axis=mybir.AxisListType.C)
                    # broadcast sums to [128, jc, i]: row (2jc + p//64)
                    sc_d = pdram.tile([1, G * 128], F32, tag="scr")
                    nc.sync.dma_start(sc_d[:], sums[:])
                    st = pstat.tile([128, NCHUNK, 128], F32, tag="st")
                    s4 = sc_d[:].rearrange("o (jc hg i) -> (o hg) jc i",
                                           hg=2, i=128)
                    for hg in range(2):
                        src = s4[hg:hg + 1].broadcast_to((64, NCHUNK, 128))
                        nc.sync.dma_start(st[hg * 64:(hg + 1) * 64, :, :], src)
                    rcp_t = pstat.tile([128, NCHUNK, 128], F32, tag="rcpt")
                    nc.vector.reciprocal(rcp_t[:], st[:])
                    OT_ps = psb.tile([128, NCHUNK, 128], F32, tag="b")
                    for g in range(G):
                        p0 = 64 * (g % 2)
                        nc.tensor.matmul(
                            OT_ps[p0:p0 + 64, g // 2, :],
                            lhsT=V[:, s, 64 * g:64 * (g + 1)],
                            rhs=AT[:, g, :],
                            start=True, stop=True)
                    OT_sb = pot.tile([128, NCHUNK, 128], F32R, tag="ot")
                    nc.vector.tensor_mul(OT_sb[:], OT_ps[:], rcp_t[:])
                    O_ps = psb.tile([128, C], F32, tag="b")
                    for jc in range(NCHUNK):
                        nc.tensor.matmul(
                            O_ps[:], lhsT=OT_sb[:, jc, :],
                            rhs=wo_sb[:, jc, :],
                            start=(jc == 0), stop=(jc == NCHUNK - 1))
                    nc.vector.tensor_add(O_sb[:, s, :], O_ps[:], bo_sb[:])
                nc.sync.dma_start(out_d[:, b * BLK:(b + 1) * BLK, :], O_sb[:])

            def back_st2(b):
                # S^T-direct: exp(S^T) IS A^T. Softmax sums via all-ones
                # matmul (broadcasts column sums to every partition in PSUM);
                # normalization folded into the OT psum->sbuf copy.
                QT, KT, V = state.pop(b)
                O_sb = po.tile([128, BLK, C], F32, tag="o")
                for s in range(BLK):
                    AT = pa.tile([128, G, 128], AMID, tag="at")
                    S_e = pss.tile([128, G // 2, 128], F32, tag="s")
                    S_o = pss.tile([128, G // 2, 128], F32, tag="s")
                    for g in range(G):
                        p0 = 64 * (g % 2)
                        S_ps = S_e if g % 2 == 0 else S_o
                        nc.tensor.matmul(
                            S_ps[:, g // 2, :],
                            lhsT=KT[p0:p0 + 64, g // 2, s, :],
                            rhs=QT[p0:p0 + 64, g // 2, s, :],
                            start=True, stop=True)
                    if exp_mode == 'perhead':
                        for g in range(G):
                            S_ps = S_e if g % 2 == 0 else S_o
                            nc.scalar.activation(
                                AT[:, g, :], S_ps[:, g // 2, :], EXP,
                                scale=1.0 / np.sqrt(GP))
                    else:
                        nc.scalar.activation(AT[:, 0:G:2, :], S_e[:], EXP,
                                             scale=1.0 / np.sqrt(GP))
                        nc.scalar.activation(AT[:, 1:G:2, :], S_o[:], EXP,
                                             scale=1.0 / np.sqrt(GP))
                    # column sums broadcast to all partitions:
                    # bc_e[p, g*128+i] = sum_j AT[j, g, i]  (heads 0-3)
                    bc_e = psb.tile([128, 512], F32, tag="b")
                    bc_o = psb.tile([128, 512], F32, tag="b")
                    nc.tensor.matmul(bc_e[:], lhsT=ones_bf[:],
                                     rhs=AT[:, 0:4, :], start=True, stop=True)
                    nc.tensor.matmul(bc_o[:], lhsT=ones_bf[:],
                                     rhs=AT[:, 4:8, :], start=True, stop=True)
                    # rcp_t[p, jc, i] = 1/sums[2jc + p//64, i]
                    rcp_t = pstat.tile([128, NCHUNK, 128], F32, tag="rcpt")
                    for hg in range(2):
                        pr = slice(hg * 64, (hg + 1) * 64)
                        for hc, bc in ((0, bc_e), (1, bc_o)):
                            # [64, 2, 128] view: cols hg*128 + jc*256
                            src = bc[pr, :].rearrange(
                                "p (jc r i) -> p jc r i", jc=2, i=128)[
                                :, :, hg, :]
                            nc.vector.reciprocal(
                                rcp_t[pr, 2 * hc:2 * hc + 2, :], src)
                    OT_ps = psb.tile([128, NCHUNK, 128], F32, tag="b")
                    for g in range(G):
                        p0 = 64 * (g % 2)
                        nc.tensor.matmul(
                            OT_ps[p0:p0 + 64, g // 2, :],
                            lhsT=V[:, s, 64 * g:64 * (g + 1)],
                            rhs=AT[:, g, :],
                            start=True, stop=True)
                    OT_sb = pot.tile([128, NCHUNK, 128], F32R, tag="ot")
                    nc.vector.tensor_mul(OT_sb[:], OT_ps[:], rcp_t[:])
                    O_ps = psb.tile([128, C], F32, tag="b")
                    for jc in range(NCHUNK):
                        nc.tensor.matmul(
                            O_ps[:], lhsT=OT_sb[:, jc, :],
                            rhs=wo_sb[:, jc, :],
                            start=(jc == 0), stop=(jc == NCHUNK - 1))
                    nc.vector.tensor_add(O_sb[:, s, :], O_ps[:], bo_sb[:])
                nc.sync.dma_start(out_d[:, b * BLK:(b + 1) * BLK, :], O_sb[:])

            back_fn = {'st': back_st, 'st2': back_st2}.get(attn_mode, back)

            def whole():
                for b in range(nblk + 1):
                    if b < nblk:
                        front(b)
                    if b >= 1:
                        back_fn(b - 1)

            if reps == 1:
                whole()
            else:
                with tc.For_i(0, reps, 1):
                    whole()

    nc.compile()
    return nc


def make_in_maps(x, Wq, Wk, Wv, Wo, bo, num_cores=8):
    """Full inputs -> per-core input dicts (data-parallel over N)."""
    x = np.asarray(x, dtype=np.float32)
    ident = np.eye(128, dtype=np.float32)
    wqT = np.ascontiguousarray(np.asarray(Wq, np.float32).T)
    wkT = np.ascontiguousarray(np.asarray(Wk, np.float32).T)
    wvT = np.ascontiguousarray(np.asarray(Wv, np.float32).T)
    woT = np.ascontiguousarray(np.asarray(Wo, np.float32).T)
    bo = np.asarray(bo, np.float32)
    return [{"x": np.ascontiguousarray(x[n]), "wqT": wqT, "wkT": wkT,
             "wvT": wvT, "woT": woT, "bo": bo, "ident": ident}
            for n in range(num_cores)]


_NC_CACHE = {}


def kernel(x, Wq, Wk, Wv, Wo, bo):
    import numpy as np
    from concourse import bass_utils

    if "nc" not in _NC_CACHE:
        _NC_CACHE["nc"] = build_kernel(
            num_cores=8, attn_mode="std", psum_bufs=(2, 3, 3),
            sbufs={"px": 3, "pxt": 3, "pqt": 3, "pv": 3, "pa": 4,
                   "pstat": 6, "pot": 3, "po": 3})
    nc = _NC_CACHE["nc"]
    in_maps = make_in_maps(x, Wq, Wk, Wv, Wo, bo, num_cores=8)
    res = bass_utils.run_bass_kernel_spmd(nc, in_maps, core_ids=list(range(8)))
    out = np.stack([res.results[c]["out"] for c in range(8)], axis=0)
    return out.astype(np.float32)



# revision 23
# speedup vs baseline: 18.7765x; 18.7765x over previous
"""Self-contained TRN2 Bass kernel for axial attention (nn_AxialAttention).

kernel(**inputs) takes FULL inputs (x [8,128,128,512], Wq/Wk/Wv/Wo [512,512],
bo [512]) and returns the FULL output [8,128,128,512] (float32).

Strategy: data-parallel over N across 8 NeuronCores (core c computes image c).
Per core: projections/output matmul at 1 cycle/row, bf16 attention middle,
softmax without max-subtraction (logits ~N(0,1)), PE transposes for X^T and
A^T. Engine balance: ACT does batched exp + selected PSUM exits, Pool does
softmax sums + A normalization, DVE does the remaining PSUM exits.
"""
import sys
sys.path.insert(0, "/opt/trn_rl_repo")
sys.path.insert(0, "/root/.axon_site/_ro/trn_rl_repo")

import numpy as np

import concourse.bass as bass
import concourse.bacc as bacc
import concourse.tile as tile
from concourse import mybir
from concourse import bass_isa

F32 = mybir.dt.float32
F32R = mybir.dt.float32r
BF16 = mybir.dt.bfloat16
EXP = mybir.ActivationFunctionType.Exp
X_AXIS = mybir.AxisListType.X

H = 128   # tokens per sequence (attention axis)
W = 128   # sequences per core
C = 512
G = 8     # heads
GP = C // G  # 64
BLK = 4   # sequences per block
NBLK = W // BLK
NCHUNK = C // 128  # 4 k-chunks


def build_kernel(num_cores=8, attn_f32=False, w_total=W, reps=1,
                 an_engine='dve', at_engine='dve', psum_bufs=(3, 2, 3),
                 exp_mode='perhead', attn_mode='std', sbufs=None,
                 psum_unified=False, exits=None, xt_bf16=False,
                 sums='bc', xt_mode='pe', xdma='sp', out_split=1,
                 norm='rcp', o_exit='add_dve'):
    """Build + compile the Bass module. Returns nc.

    exits: dict mapping exit-copy site -> engine ('dve'|'act') for the
    PSUM->SBUF copies: keys 'xt', 'q', 'k', 'v', 'ot'.
    exp_mode: 'perhead' (8 ACT instrs w/ accum), 'batched' (2 ACT instrs,
    sums on DVE), 'batched_pool' (2 ACT instrs, sums on Pool).
    an_engine: 'dve'|'pool' for the A*(1/sum) normalization multiply.
    xt_bf16: convert X to bf16 before the PE transposes (1.0 vs 1.5
    cycles/row) and keep Wq/Wk/Wv in bf16.
    """
    nblk = w_total // BLK
    ex = {'xt': 'dve', 'q': 'dve', 'k': 'dve', 'v': 'dve', 'ot': 'dve'}
    ex.update(exits or {})
    if xt_mode in ('dma', 'pebf'):
        xt_bf16 = True
    nc = bacc.Bacc("TRN2", target_bir_lowering=False, debug=False,
                   num_devices=num_cores)

    XIN = BF16 if xt_mode in ('dma', 'pebf') else F32R
    WIN = BF16 if xt_mode in ('dma', 'pebf') else F32R
    x_d = nc.dram_tensor("x", [H, w_total, C], XIN, kind="ExternalInput").ap()
    wq_d = nc.dram_tensor("wqT", [C, C], WIN, kind="ExternalInput").ap()
    wk_d = nc.dram_tensor("wkT", [C, C], WIN, kind="ExternalInput").ap()
    wv_d = nc.dram_tensor("wvT", [C, C], WIN, kind="ExternalInput").ap()
    wo_d = nc.dram_tensor("woT", [C, C], F32R, kind="ExternalInput").ap()
    bo_d = nc.dram_tensor("bo", [C], F32, kind="ExternalInput").ap()
    id_d = nc.dram_tensor("ident", [128, 128], F32R, kind="ExternalInput").ap()
    out_d = nc.dram_tensor("out", [H, w_total, C], F32, kind="ExternalOutput").ap()

    AMID = F32R if attn_f32 else BF16  # dtype of attention middle section
    XDT = BF16 if xt_bf16 else F32R    # dtype of X^T / projection inputs

    def exit_copy(site, dst, src):
        if ex[site] == 'act':
            nc.scalar.copy(dst, src)
        else:
            nc.vector.tensor_copy(dst, src)

    sb_bufs = sbufs or {}
    def B(name, d):
        return sb_bufs.get(name, d)
    with tile.TileContext(nc) as tc:
        with tc.tile_pool(name="consts", bufs=1) as consts, \
             tc.tile_pool(name="px", bufs=B('px', 2)) as px, \
             tc.tile_pool(name="pxt", bufs=B('pxt', 2)) as pxt, \
             tc.tile_pool(name="pqt", bufs=B('pqt', 2)) as pqt, \
             tc.tile_pool(name="pv", bufs=B('pv', 2)) as pv, \
             tc.tile_pool(name="pa", bufs=B('pa', 3)) as pa, \
             tc.tile_pool(name="pstat", bufs=B('pstat', 4)) as pstat, \
             tc.tile_pool(name="pot", bufs=B('pot', 2)) as pot, \
             tc.tile_pool(name="po", bufs=B('po', 2)) as po, \
             tc.tile_pool(name="pdram", bufs=6, space="DRAM") as pdram, \
             tc.tile_pool(name="psf", bufs=(8 if psum_unified else psum_bufs[0]), space="PSUM") as psf, \
             tc.tile_pool(name="pss", bufs=psum_bufs[1], space="PSUM") as _pss, \
             tc.tile_pool(name="psb", bufs=psum_bufs[2], space="PSUM") as _psb:
            if psum_unified:
                class _U:
                    _n = [0]
                    def tile(self, shape, dtype, tag=None):
                        self._n[0] += 1
                        return psf.tile(shape, dtype, tag="f",
                                        name=f"u{self._n[0]}")
                pss = psb = _U()
            else:
                pss, psb = _pss, _psb

            # ---- constants ----
            # DMA issue order matters (SP queue is FIFO): identity first
            # (needed by the first transposes), then Wq/Wk (first matmuls),
            # then Wv, then Wo/bo (only needed by back(0), ~15us later).
            id_sb = consts.tile([128, 128], F32R, tag="id")
            nc.sync.dma_start(id_sb[:], id_d[:])
            id_bf = consts.tile([128, 128], BF16, tag="idbf")
            nc.vector.tensor_copy(id_bf[:], id_sb[:].bitcast(F32))
            wq_sb = consts.tile([128, NCHUNK, C], XDT, tag="wq")
            wk_sb = consts.tile([128, NCHUNK, C], XDT, tag="wk")
            wv_sb = consts.tile([128, NCHUNK, C], XDT, tag="wv")
            wo_sb = consts.tile([128, NCHUNK, C], F32R, tag="wo")
            if xt_mode in ('dma', 'pebf'):
                # weights arrive pre-converted to bf16 from the host
                for w_sb, w_d in ((wq_sb, wq_d), (wk_sb, wk_d),
                                  (wv_sb, wv_d)):
                    nc.sync.dma_start(
                        w_sb[:], w_d.rearrange("(j p) c -> p j c", p=128))
            elif xt_bf16:
                wtmp = consts.tile([128, 3, NCHUNK, C], F32R, tag="wtmp")
                for i, w_d in enumerate((wq_d, wk_d, wv_d)):
                    nc.sync.dma_start(
                        wtmp[:, i, :, :],
                        w_d.rearrange("(j p) c -> p j c", p=128))
                for i, w_sb in enumerate((wq_sb, wk_sb, wv_sb)):
                    nc.vector.tensor_copy(w_sb[:],
                                          wtmp[:, i, :, :].bitcast(F32))
            else:
                for w_sb, w_d in ((wq_sb, wq_d), (wk_sb, wk_d),
                                  (wv_sb, wv_d)):
                    nc.sync.dma_start(
                        w_sb[:], w_d.rearrange("(j p) c -> p j c", p=128))
            nc.sync.dma_start(wo_sb[:],
                              wo_d.rearrange("(j p) c -> p j c", p=128))
            bo_sb = consts.tile([128, C], F32, tag="bo")
            nc.sync.dma_start(
                bo_sb[:],
                bo_d.rearrange("(o c) -> o c", o=1).broadcast_to((128, C)))
            ones_bf = consts.tile([128, 128], BF16, tag="ones")
            nc.vector.memset(ones_bf[:], 1.0)
            id_x = id_bf if xt_bf16 else id_sb

            state = {}

            def front(b):
                XT_sb = pxt.tile([128, NCHUNK, BLK, 128], XDT, tag="xt")
                if xt_mode == 'dma':
                    # xbar DMA transpose straight from DRAM (x is bf16):
                    # [128 h, 128 c] -> [128 c, 128 h] per (seq, chunk).
                    for s in range(BLK):
                        for jc in range(NCHUNK):
                            nc.sync.dma_start_transpose(
                                XT_sb[:, jc, s, :],
                                x_d[:, b * BLK + s,
                                    jc * 128:(jc + 1) * 128])
                else:
                    X_blk = px.tile([128, BLK, C], XIN, tag="x")
                    xq = nc.scalar if xdma == 'act' else nc.sync
                    xq.dma_start(X_blk[:],
                                 x_d[:, b * BLK:(b + 1) * BLK, :])
                    if xt_bf16 and xt_mode != 'pebf':
                        X_in = px.tile([128, BLK, C], BF16, tag="xbf")
                        nc.vector.tensor_copy(X_in[:], X_blk[:].bitcast(F32))
                    else:
                        X_in = X_blk
                    for s in range(BLK):
                        XT_ps = psf.tile([128, NCHUNK, 128], XDT, tag="f")
                        for jc in range(NCHUNK):
                            nc.tensor.transpose(
                                XT_ps[:, jc, :],
                                X_in[:, s, jc * 128:(jc + 1) * 128], id_x[:])
                        exit_copy('xt', XT_sb[:, :, s, :], XT_ps[:])
                QT = pqt.tile([128, NCHUNK, BLK, 128], AMID, tag="qt")
                KT = pqt.tile([128, NCHUNK, BLK, 128], AMID, tag="kt")
                for w_sb, dst, site in ((wq_sb, QT, 'q'), (wk_sb, KT, 'k')):
                    for co in range(NCHUNK):
                        PT = psf.tile([128, BLK * 128], F32, tag="f")
                        for jc in range(NCHUNK):
                            nc.tensor.matmul(
                                PT[:],
                                lhsT=w_sb[:, jc, co * 128:(co + 1) * 128],
                                rhs=XT_sb[:, jc, :, :],
                                start=(jc == 0), stop=(jc == NCHUNK - 1))
                        exit_copy(site, dst[:, co, :, :], PT[:])
                V = pv.tile([128, BLK, C], AMID, tag="v")
                for s in range(BLK):
                    VP = psf.tile([128, C], F32, tag="f")
                    for jc in range(NCHUNK):
                        nc.tensor.matmul(
                            VP[:], lhsT=XT_sb[:, jc, s, :],
                            rhs=wv_sb[:, jc, :],
                            start=(jc == 0), stop=(jc == NCHUNK - 1))
                    exit_copy('v', V[:, s, :], VP[:])
                state[b] = (QT, KT, V)

            def back(b):
                QT, KT, V = state.pop(b)
                O_sb = po.tile([128, BLK, C], F32, tag="o")
                for s in range(BLK):
                    A = pa.tile([128, G, 128], AMID, tag="a")
                    sums = pstat.tile([128, G], F32, tag="sums")
                    # Even heads (PE row-group 0) and odd heads (row-group 1)
                    # run concurrently in the array -> MUST land in different
                    # PSUM banks (same-bank concurrent row-group writes hang).
                    S_e = pss.tile([128, G // 2, 128], F32, tag="s")
                    S_o = pss.tile([128, G // 2, 128], F32, tag="s")
                    for g in range(G):
                        p0 = 64 * (g % 2)
                        S_ps = S_e if g % 2 == 0 else S_o
                        nc.tensor.matmul(
                            S_ps[:, g // 2, :],
                            lhsT=QT[p0:p0 + 64, g // 2, s, :],
                            rhs=KT[p0:p0 + 64, g // 2, s, :],
                            start=True, stop=True)
                    if exp_mode == 'perhead':
                        for g in range(G):
                            S_ps = S_e if g % 2 == 0 else S_o
                            nc.scalar.activation(
                                A[:, g, :], S_ps[:, g // 2, :], EXP,
                                scale=1.0 / np.sqrt(GP),
                                accum_out=sums[:, g:g + 1])
                    else:
                        # batched: 2 exps; A slices by even/odd heads; sums
                        # via reduce over [128, 4, 128] per half on DVE/Pool
                        a_ev = A[:, 0:G:2, :]
                        a_od = A[:, 1:G:2, :]
                        nc.scalar.activation(a_ev, S_e[:], EXP,
                                             scale=1.0 / np.sqrt(GP))
                        nc.scalar.activation(a_od, S_o[:], EXP,
                                             scale=1.0 / np.sqrt(GP))
                        se = sums[:, 0:G:2].rearrange("p (g o) -> p g o", o=1)
                        so = sums[:, 1:G:2].rearrange("p (g o) -> p g o", o=1)
                        red = (nc.gpsimd if exp_mode == 'batched_pool'
                               else nc.vector)
                        red.reduce_sum(se, a_ev, axis=X_AXIS)
                        red.reduce_sum(so, a_od, axis=X_AXIS)
                    rcp = pstat.tile([128, G], F32, tag="rcp")
                    nc.vector.reciprocal(rcp[:], sums[:])
                    AN = pa.tile([128, G, 128], AMID, tag="an")
                    rcp_b = rcp[:].rearrange("p (o g) -> p o g", o=1) \
                        .rearrange("p o g -> p g o") \
                        .broadcast_to((128, G, 128))
                    if an_engine == 'pool':
                        nc.gpsimd.tensor_mul(AN[:], A[:], rcp_b)
                    else:
                        nc.vector.tensor_mul(AN[:], A[:], rcp_b)
                    # A^T via PE transposes (xbar DMA transpose races with
                    # concurrent DRAM DMA traffic on this HW/runtime).
                    AT = pa.tile([128, G, 128], AMID, tag="at")
                    idt = id_bf if AMID == BF16 else id_sb
                    for half in range(2):
                        ATp = psb.tile([128, G // 2, 128], AMID, tag="b")
                        for gg in range(G // 2):
                            g = half * (G // 2) + gg
                            nc.tensor.transpose(ATp[:, gg, :], AN[:, g, :],
                                                idt[:])
                        if at_engine == 'dve':
                            nc.vector.tensor_copy(
                                AT[:, half * (G // 2):(half + 1) * (G // 2), :],
                                ATp[:])
                        else:
                            nc.scalar.copy(
                                AT[:, half * (G // 2):(half + 1) * (G // 2), :],
                                ATp[:])
                    OT_ps = psb.tile([128, NCHUNK, 128], F32, tag="b")
                    for g in range(G):
                        p0 = 64 * (g % 2)
                        nc.tensor.matmul(
                            OT_ps[p0:p0 + 64, g // 2, :],
                            lhsT=V[:, s, 64 * g:64 * (g + 1)],
                            rhs=AT[:, g, :],
                            start=True, stop=True)
                    OT_sb = pot.tile([128, NCHUNK, 128], F32R, tag="ot")
                    exit_copy('ot', OT_sb[:], OT_ps[:])
                    O_ps = psb.tile([128, C], F32, tag="b")
                    for jc in range(NCHUNK):
                        nc.tensor.matmul(
                            O_ps[:], lhsT=OT_sb[:, jc, :],
                            rhs=wo_sb[:, jc, :],
                            start=(jc == 0), stop=(jc == NCHUNK - 1))
                    if o_exit == 'copy_act':
                        nc.scalar.copy(O_sb[:, s, :], O_ps[:])
                    elif o_exit == 'copy_dve':
                        nc.vector.tensor_copy(O_sb[:, s, :], O_ps[:])
                    else:
                        nc.vector.tensor_add(O_sb[:, s, :], O_ps[:], bo_sb[:])
                for o0 in range(0, BLK, BLK // out_split):
                    o1 = o0 + BLK // out_split
                    nc.sync.dma_start(
                        out_d[:, b * BLK + o0:b * BLK + o1, :],
                        O_sb[:, o0:o1, :])

            def back_st2(b):
                # S^T-direct: compute S^T (lhsT/rhs swapped), so exp(S^T) IS
                # the unnormalized A^T — no A-normalize multiply, no PE
                # transposes of A, no A^T psum exits. Softmax sums come from
                # all-ones matmuls (column sums broadcast to every partition
                # in PSUM); normalization is folded into the OT psum exit.
                QT, KT, V = state.pop(b)
                O_sb = po.tile([128, BLK, C], F32, tag="o")
                for s in range(BLK):
                    AT = pa.tile([128, G, 128], AMID, tag="at")
                    S_e = pss.tile([128, G // 2, 128], F32, tag="s")
                    S_o = pss.tile([128, G // 2, 128], F32, tag="s")
                    for g in range(G):
                        p0 = 64 * (g % 2)
                        S_ps = S_e if g % 2 == 0 else S_o
                        # lhsT=K^T, rhs=Q^T  ->  S^T[j, i]
                        nc.tensor.matmul(
                            S_ps[:, g // 2, :],
                            lhsT=KT[p0:p0 + 64, g // 2, s, :],
                            rhs=QT[p0:p0 + 64, g // 2, s, :],
                            start=True, stop=True)
                    # exp: AT[:, g, :] = exp(S^T_g / sqrt(gp)); even heads
                    # from S_e, odd from S_o (two batched ACT instructions).
                    nc.scalar.activation(AT[:, 0:G:2, :], S_e[:], EXP,
                                         scale=1.0 / np.sqrt(GP))
                    nc.scalar.activation(AT[:, 1:G:2, :], S_o[:], EXP,
                                         scale=1.0 / np.sqrt(GP))
                    if sums == 'allreduce':
                        # Pool cross-partition all-reduce: every partition
                        # gets all 8 heads' column sums; normalize A^T on
                        # Pool (the partition-replicated sums operand is
                        # affine) either via DVE reciprocal + Pool multiply
                        # or a direct Pool divide.
                        sums_bc = pstat.tile([128, G, 128], F32, tag="sbc")
                        nc.gpsimd.partition_all_reduce(
                            sums_bc[:], AT[:], 128, bass_isa.ReduceOp.add)
                        ATN = pa.tile([128, G, 128], AMID, tag="atn")
                        if norm == 'div':
                            nc.gpsimd.tensor_tensor(
                                ATN[:], AT[:], sums_bc[:],
                                op=mybir.AluOpType.divide)
                        else:
                            rcp_f = pstat.tile([128, G, 128], F32, tag="rcpf")
                            nc.vector.reciprocal(rcp_f[:], sums_bc[:])
                            nc.gpsimd.tensor_mul(ATN[:], AT[:], rcp_f[:])
                        OT_ps = psb.tile([128, NCHUNK, 128], F32, tag="b")
                        for g in range(G):
                            p0 = 64 * (g % 2)
                            nc.tensor.matmul(
                                OT_ps[p0:p0 + 64, g // 2, :],
                                lhsT=V[:, s, 64 * g:64 * (g + 1)],
                                rhs=ATN[:, g, :],
                                start=True, stop=True)
                        OT_sb = pot.tile([128, NCHUNK, 128], F32R, tag="ot")
                        exit_copy('ot', OT_sb[:], OT_ps[:])
                    else:
                        # column sums broadcast to all partitions:
                        # bc_e[p, q*128+i] = sum_j AT[j, 2q, i]   (even)
                        # bc_o[p, q*128+i] = sum_j AT[j, 2q+1, i] (odd)
                        bc_e = psb.tile([128, NCHUNK, 128], F32, tag="b")
                        bc_o = psb.tile([128, NCHUNK, 128], F32, tag="b")
                        nc.tensor.matmul(bc_e[:], lhsT=ones_bf[:],
                                         rhs=AT[:, 0:G:2, :], start=True,
                                         stop=True)
                        nc.tensor.matmul(bc_o[:], lhsT=ones_bf[:],
                                         rhs=AT[:, 1:G:2, :], start=True,
                                         stop=True)
                        # OT_ps[64*(g%2)+c, g//2, i] has head g=2q+(p>=64):
                        # top partitions read even-head sums, bottom odd.
                        rcp_t = pstat.tile([128, NCHUNK, 128], F32,
                                           tag="rcpt")
                        nc.vector.reciprocal(rcp_t[0:64, :, :],
                                             bc_e[0:64, :, :])
                        nc.vector.reciprocal(rcp_t[64:128, :, :],
                                             bc_o[64:128, :, :])
                        OT_ps = psb.tile([128, NCHUNK, 128], F32, tag="b")
                        for g in range(G):
                            p0 = 64 * (g % 2)
                            nc.tensor.matmul(
                                OT_ps[p0:p0 + 64, g // 2, :],
                                lhsT=V[:, s, 64 * g:64 * (g + 1)],
                                rhs=AT[:, g, :],
                                start=True, stop=True)
                        OT_sb = pot.tile([128, NCHUNK, 128], F32R, tag="ot")
                        nc.vector.tensor_mul(OT_sb[:], OT_ps[:], rcp_t[:])
                    O_ps = psb.tile([128, C], F32, tag="b")
                    for jc in range(NCHUNK):
                        nc.tensor.matmul(
                            O_ps[:], lhsT=OT_sb[:, jc, :],
                            rhs=wo_sb[:, jc, :],
                            start=(jc == 0), stop=(jc == NCHUNK - 1))
                    if o_exit == 'copy_act':
                        nc.scalar.copy(O_sb[:, s, :], O_ps[:])
                    elif o_exit == 'copy_dve':
                        nc.vector.tensor_copy(O_sb[:, s, :], O_ps[:])
                    else:
                        nc.vector.tensor_add(O_sb[:, s, :], O_ps[:], bo_sb[:])
                for o0 in range(0, BLK, BLK // out_split):
                    o1 = o0 + BLK // out_split
                    nc.sync.dma_start(
                        out_d[:, b * BLK + o0:b * BLK + o1, :],
                        O_sb[:, o0:o1, :])

            back_fn = back_st2 if attn_mode == 'st2' else back

            def whole():
                for b in range(nblk + 1):
                    if b < nblk:
                        front(b)
                    if b >= 1:
                        back_fn(b - 1)

            if reps == 1:
                whole()
            else:
                with tc.For_i(0, reps, 1):
                    whole()

    nc.compile()
    return nc


def make_in_maps(x, Wq, Wk, Wv, Wo, bo, num_cores=8, x_bf16=False):
    """Full inputs -> per-core input dicts (data-parallel over N)."""
    x = np.asarray(x, dtype=np.float32)
    wdt = np.float32
    if x_bf16:
        import ml_dtypes
        x = x.astype(ml_dtypes.bfloat16)
        wdt = ml_dtypes.bfloat16
    ident = np.eye(128, dtype=np.float32)
    wqT = np.ascontiguousarray(np.asarray(Wq, np.float32).T.astype(wdt))
    wkT = np.ascontiguousarray(np.asarray(Wk, np.float32).T.astype(wdt))
    wvT = np.ascontiguousarray(np.asarray(Wv, np.float32).T.astype(wdt))
    woT = np.ascontiguousarray(np.asarray(Wo, np.float32).T)
    bo = np.asarray(bo, np.float32)
    return [{"x": np.ascontiguousarray(x[n]), "wqT": wqT, "wkT": wkT,
             "wvT": wvT, "woT": woT, "bo": bo, "ident": ident}
            for n in range(num_cores)]


BEST_CONFIG = dict(
    attn_mode='std', psum_bufs=(2, 3, 3),
    sbufs={"px": 3, "pxt": 3, "pqt": 3, "pv": 3, "pa": 4,
           "pstat": 6, "pot": 3, "po": 3})

_NC_CACHE = {}


def kernel(x, Wq, Wk, Wv, Wo, bo):
    import numpy as np
    from concourse import bass_utils

    if "nc" not in _NC_CACHE:
        _NC_CACHE["nc"] = build_kernel(num_cores=8, **BEST_CONFIG)
    nc = _NC_CACHE["nc"]
    in_maps = make_in_maps(x, Wq, Wk, Wv, Wo, bo, num_cores=8)
    res = bass_utils.run_bass_kernel_spmd(nc, in_maps, core_ids=list(range(8)))
    out = np.stack([res.results[c]["out"] for c in range(8)], axis=0)
    return out.astype(np.float32)


# revision 27
# speedup vs baseline: 22.1676x; 1.1806x over previous
"""Self-contained TRN2 Bass kernel for axial attention (nn_AxialAttention).

kernel(**inputs) takes FULL inputs (x [8,128,128,512], Wq/Wk/Wv/Wo [512,512],
bo [512]) and returns the FULL output [8,128,128,512] (float32).

Strategy: data-parallel over N across 8 NeuronCores (core c computes image c).
Per core (best config): bf16 projections at 1 cycle/row; X^T produced by
xbar DMA transposes straight from DRAM (isolated on the SP queue — mixing
xbar and normal descriptors on one queue corrupts results; all other DMA
rides the ACT queue); S^T-direct attention (lhsT/rhs swapped so exp(S^T) IS
the unnormalized A^T — no A transposes or normalization multiply on A);
softmax without max-subtraction (logits ~N(0,1)); column sums broadcast via
all-ones matmuls; normalization folded into the OT PSUM exit. Engine
balance: ACT runs batched exp + Q/V PSUM exits, DVE the rest; Pool/GpSimd
deliberately unused (much slower on HW than the cost model suggests).
"""
import sys
sys.path.insert(0, "/opt/trn_rl_repo")
sys.path.insert(0, "/root/.axon_site/_ro/trn_rl_repo")

import numpy as np

import concourse.bass as bass
import concourse.bacc as bacc
import concourse.tile as tile
from concourse import mybir
from concourse import bass_isa

F32 = mybir.dt.float32
F32R = mybir.dt.float32r
BF16 = mybir.dt.bfloat16
EXP = mybir.ActivationFunctionType.Exp
X_AXIS = mybir.AxisListType.X

H = 128   # tokens per sequence (attention axis)
W = 128   # sequences per core
C = 512
G = 8     # heads
GP = C // G  # 64
BLK = 4   # sequences per block
NBLK = W // BLK
NCHUNK = C // 128  # 4 k-chunks


def build_kernel(num_cores=8, attn_f32=False, w_total=W, reps=1,
                 an_engine='dve', at_engine='dve', psum_bufs=(3, 2, 3),
                 exp_mode='perhead', attn_mode='std', sbufs=None,
                 psum_unified=False, exits=None, xt_bf16=False,
                 sums='bc', xt_mode='pe', xdma='sp', out_split=1,
                 norm='rcp', o_exit='add_dve', odma='sp', wdma='sp'):
    """Build + compile the Bass module. Returns nc.

    exits: dict mapping exit-copy site -> engine ('dve'|'act') for the
    PSUM->SBUF copies: keys 'xt', 'q', 'k', 'v', 'ot'.
    exp_mode: 'perhead' (8 ACT instrs w/ accum), 'batched' (2 ACT instrs,
    sums on DVE), 'batched_pool' (2 ACT instrs, sums on Pool).
    an_engine: 'dve'|'pool' for the A*(1/sum) normalization multiply.
    xt_bf16: convert X to bf16 before the PE transposes (1.0 vs 1.5
    cycles/row) and keep Wq/Wk/Wv in bf16.
    """
    nblk = w_total // BLK
    ex = {'xt': 'dve', 'q': 'dve', 'k': 'dve', 'v': 'dve', 'ot': 'dve'}
    ex.update(exits or {})
    if xt_mode in ('dma', 'pebf'):
        xt_bf16 = True
    nc = bacc.Bacc("TRN2", target_bir_lowering=False, debug=False,
                   num_devices=num_cores)

    XIN = BF16 if xt_mode in ('dma', 'pebf') else F32R
    WIN = BF16 if xt_mode in ('dma', 'pebf') else F32R
    x_d = nc.dram_tensor("x", [H, w_total, C], XIN, kind="ExternalInput").ap()
    wq_d = nc.dram_tensor("wqT", [C, C], WIN, kind="ExternalInput").ap()
    wk_d = nc.dram_tensor("wkT", [C, C], WIN, kind="ExternalInput").ap()
    wv_d = nc.dram_tensor("wvT", [C, C], WIN, kind="ExternalInput").ap()
    wo_d = nc.dram_tensor("woT", [C, C], F32R, kind="ExternalInput").ap()
    bo_d = nc.dram_tensor("bo", [C], F32, kind="ExternalInput").ap()
    id_d = nc.dram_tensor("ident", [128, 128], F32R, kind="ExternalInput").ap()
    out_d = nc.dram_tensor("out", [H, w_total, C], F32, kind="ExternalOutput").ap()

    AMID = F32R if attn_f32 else BF16  # dtype of attention middle section
    XDT = BF16 if xt_bf16 else F32R    # dtype of X^T / projection inputs

    def exit_copy(site, dst, src):
        if ex[site] == 'act':
            nc.scalar.copy(dst, src)
        else:
            nc.vector.tensor_copy(dst, src)

    sb_bufs = sbufs or {}
    def B(name, d):
        return sb_bufs.get(name, d)
    with tile.TileContext(nc) as tc:
        with tc.tile_pool(name="consts", bufs=1) as consts, \
             tc.tile_pool(name="px", bufs=B('px', 2)) as px, \
             tc.tile_pool(name="pxt", bufs=B('pxt', 2)) as pxt, \
             tc.tile_pool(name="pqt", bufs=B('pqt', 2)) as pqt, \
             tc.tile_pool(name="pv", bufs=B('pv', 2)) as pv, \
             tc.tile_pool(name="pa", bufs=B('pa', 3)) as pa, \
             tc.tile_pool(name="pstat", bufs=B('pstat', 4)) as pstat, \
             tc.tile_pool(name="pot", bufs=B('pot', 2)) as pot, \
             tc.tile_pool(name="po", bufs=B('po', 2)) as po, \
             tc.tile_pool(name="pdram", bufs=6, space="DRAM") as pdram, \
             tc.tile_pool(name="psf", bufs=(8 if psum_unified else psum_bufs[0]), space="PSUM") as psf, \
             tc.tile_pool(name="pss", bufs=psum_bufs[1], space="PSUM") as _pss, \
             tc.tile_pool(name="psb", bufs=psum_bufs[2], space="PSUM") as _psb:
            if psum_unified:
                class _U:
                    _n = [0]
                    def tile(self, shape, dtype, tag=None):
                        self._n[0] += 1
                        return psf.tile(shape, dtype, tag="f",
                                        name=f"u{self._n[0]}")
                pss = psb = _U()
            else:
                pss, psb = _pss, _psb

            # ---- constants ----
            # DMA issue order matters (SP queue is FIFO): identity first
            # (needed by the first transposes), then Wq/Wk (first matmuls),
            # then Wv, then Wo/bo (only needed by back(0), ~15us later).
            wq_eng = nc.scalar if wdma == 'act' else nc.sync
            id_sb = consts.tile([128, 128], F32R, tag="id")
            wq_eng.dma_start(id_sb[:], id_d[:])
            id_bf = consts.tile([128, 128], BF16, tag="idbf")
            nc.vector.tensor_copy(id_bf[:], id_sb[:].bitcast(F32))
            wq_sb = consts.tile([128, NCHUNK, C], XDT, tag="wq")
            wk_sb = consts.tile([128, NCHUNK, C], XDT, tag="wk")
            wv_sb = consts.tile([128, NCHUNK, C], XDT, tag="wv")
            wo_sb = consts.tile([128, NCHUNK, C], F32R, tag="wo")
            if xt_mode in ('dma', 'pebf'):
                # weights arrive pre-converted to bf16 from the host
                for w_sb, w_d in ((wq_sb, wq_d), (wk_sb, wk_d),
                                  (wv_sb, wv_d)):
                    wq_eng.dma_start(
                        w_sb[:], w_d.rearrange("(j p) c -> p j c", p=128))
            elif xt_bf16:
                wtmp = consts.tile([128, 3, NCHUNK, C], F32R, tag="wtmp")
                for i, w_d in enumerate((wq_d, wk_d, wv_d)):
                    nc.sync.dma_start(
                        wtmp[:, i, :, :],
                        w_d.rearrange("(j p) c -> p j c", p=128))
                for i, w_sb in enumerate((wq_sb, wk_sb, wv_sb)):
                    nc.vector.tensor_copy(w_sb[:],
                                          wtmp[:, i, :, :].bitcast(F32))
            else:
                for w_sb, w_d in ((wq_sb, wq_d), (wk_sb, wk_d),
                                  (wv_sb, wv_d)):
                    nc.sync.dma_start(
                        w_sb[:], w_d.rearrange("(j p) c -> p j c", p=128))
            wq_eng.dma_start(wo_sb[:],
                             wo_d.rearrange("(j p) c -> p j c", p=128))
            bo_sb = consts.tile([128, C], F32, tag="bo")
            wq_eng.dma_start(
                bo_sb[:],
                bo_d.rearrange("(o c) -> o c", o=1).broadcast_to((128, C)))
            ones_bf = consts.tile([128, 128], BF16, tag="ones")
            nc.vector.memset(ones_bf[:], 1.0)
            id_x = id_bf if xt_bf16 else id_sb

            state = {}

            def front(b):
                XT_sb = pxt.tile([128, NCHUNK, BLK, 128], XDT, tag="xt")
                if xt_mode == 'dma':
                    # xbar DMA transpose straight from DRAM (x is bf16):
                    # [128 h, 128 c] -> [128 c, 128 h] per (seq, chunk).
                    for s in range(BLK):
                        for jc in range(NCHUNK):
                            nc.sync.dma_start_transpose(
                                XT_sb[:, jc, s, :],
                                x_d[:, b * BLK + s,
                                    jc * 128:(jc + 1) * 128])
                else:
                    X_blk = px.tile([128, BLK, C], XIN, tag="x")
                    xq = nc.scalar if xdma == 'act' else nc.sync
                    xq.dma_start(X_blk[:],
                                 x_d[:, b * BLK:(b + 1) * BLK, :])
                    if xt_bf16 and xt_mode != 'pebf':
                        X_in = px.tile([128, BLK, C], BF16, tag="xbf")
                        nc.vector.tensor_copy(X_in[:], X_blk[:].bitcast(F32))
                    else:
                        X_in = X_blk
                    for s in range(BLK):
                        XT_ps = psf.tile([128, NCHUNK, 128], XDT, tag="f")
                        for jc in range(NCHUNK):
                            nc.tensor.transpose(
                                XT_ps[:, jc, :],
                                X_in[:, s, jc * 128:(jc + 1) * 128], id_x[:])
                        exit_copy('xt', XT_sb[:, :, s, :], XT_ps[:])
                QT = pqt.tile([128, NCHUNK, BLK, 128], AMID, tag="qt")
                KT = pqt.tile([128, NCHUNK, BLK, 128], AMID, tag="kt")
                for w_sb, dst, site in ((wq_sb, QT, 'q'), (wk_sb, KT, 'k')):
                    for co in range(NCHUNK):
                        PT = psf.tile([128, BLK * 128], F32, tag="f")
                        for jc in range(NCHUNK):
                            nc.tensor.matmul(
                                PT[:],
                                lhsT=w_sb[:, jc, co * 128:(co + 1) * 128],
                                rhs=XT_sb[:, jc, :, :],
                                start=(jc == 0), stop=(jc == NCHUNK - 1))
                        exit_copy(site, dst[:, co, :, :], PT[:])
                V = pv.tile([128, BLK, C], AMID, tag="v")
                for s in range(BLK):
                    VP = psf.tile([128, C], F32, tag="f")
                    for jc in range(NCHUNK):
                        nc.tensor.matmul(
                            VP[:], lhsT=XT_sb[:, jc, s, :],
                            rhs=wv_sb[:, jc, :],
                            start=(jc == 0), stop=(jc == NCHUNK - 1))
                    exit_copy('v', V[:, s, :], VP[:])
                state[b] = (QT, KT, V)

            def back(b):
                QT, KT, V = state.pop(b)
                O_sb = po.tile([128, BLK, C], F32, tag="o")
                for s in range(BLK):
                    A = pa.tile([128, G, 128], AMID, tag="a")
                    sums = pstat.tile([128, G], F32, tag="sums")
                    # Even heads (PE row-group 0) and odd heads (row-group 1)
                    # run concurrently in the array -> MUST land in different
                    # PSUM banks (same-bank concurrent row-group writes hang).
                    S_e = pss.tile([128, G // 2, 128], F32, tag="s")
                    S_o = pss.tile([128, G // 2, 128], F32, tag="s")
                    for g in range(G):
                        p0 = 64 * (g % 2)
                        S_ps = S_e if g % 2 == 0 else S_o
                        nc.tensor.matmul(
                            S_ps[:, g // 2, :],
                            lhsT=QT[p0:p0 + 64, g // 2, s, :],
                            rhs=KT[p0:p0 + 64, g // 2, s, :],
                            start=True, stop=True)
                    if exp_mode == 'perhead':
                        for g in range(G):
                            S_ps = S_e if g % 2 == 0 else S_o
                            nc.scalar.activation(
                                A[:, g, :], S_ps[:, g // 2, :], EXP,
                                scale=1.0 / np.sqrt(GP),
                                accum_out=sums[:, g:g + 1])
                    else:
                        # batched: 2 exps; A slices by even/odd heads; sums
                        # via reduce over [128, 4, 128] per half on DVE/Pool
                        a_ev = A[:, 0:G:2, :]
                        a_od = A[:, 1:G:2, :]
                        nc.scalar.activation(a_ev, S_e[:], EXP,
                                             scale=1.0 / np.sqrt(GP))
                        nc.scalar.activation(a_od, S_o[:], EXP,
                                             scale=1.0 / np.sqrt(GP))
                        se = sums[:, 0:G:2].rearrange("p (g o) -> p g o", o=1)
                        so = sums[:, 1:G:2].rearrange("p (g o) -> p g o", o=1)
                        red = (nc.gpsimd if exp_mode == 'batched_pool'
                               else nc.vector)
                        red.reduce_sum(se, a_ev, axis=X_AXIS)
                        red.reduce_sum(so, a_od, axis=X_AXIS)
                    rcp = pstat.tile([128, G], F32, tag="rcp")
                    nc.vector.reciprocal(rcp[:], sums[:])
                    AN = pa.tile([128, G, 128], AMID, tag="an")
                    rcp_b = rcp[:].rearrange("p (o g) -> p o g", o=1) \
                        .rearrange("p o g -> p g o") \
                        .broadcast_to((128, G, 128))
                    if an_engine == 'pool':
                        nc.gpsimd.tensor_mul(AN[:], A[:], rcp_b)
                    else:
                        nc.vector.tensor_mul(AN[:], A[:], rcp_b)
                    # A^T via PE transposes (xbar DMA transpose races with
                    # concurrent DRAM DMA traffic on this HW/runtime).
                    AT = pa.tile([128, G, 128], AMID, tag="at")
                    idt = id_bf if AMID == BF16 else id_sb
                    for half in range(2):
                        ATp = psb.tile([128, G // 2, 128], AMID, tag="b")
                        for gg in range(G // 2):
                            g = half * (G // 2) + gg
                            nc.tensor.transpose(ATp[:, gg, :], AN[:, g, :],
                                                idt[:])
                        if at_engine == 'dve':
                            nc.vector.tensor_copy(
                                AT[:, half * (G // 2):(half + 1) * (G // 2), :],
                                ATp[:])
                        else:
                            nc.scalar.copy(
                                AT[:, half * (G // 2):(half + 1) * (G // 2), :],
                                ATp[:])
                    OT_ps = psb.tile([128, NCHUNK, 128], F32, tag="b")
                    for g in range(G):
                        p0 = 64 * (g % 2)
                        nc.tensor.matmul(
                            OT_ps[p0:p0 + 64, g // 2, :],
                            lhsT=V[:, s, 64 * g:64 * (g + 1)],
                            rhs=AT[:, g, :],
                            start=True, stop=True)
                    OT_sb = pot.tile([128, NCHUNK, 128], F32R, tag="ot")
                    exit_copy('ot', OT_sb[:], OT_ps[:])
                    O_ps = psb.tile([128, C], F32, tag="b")
                    for jc in range(NCHUNK):
                        nc.tensor.matmul(
                            O_ps[:], lhsT=OT_sb[:, jc, :],
                            rhs=wo_sb[:, jc, :],
                            start=(jc == 0), stop=(jc == NCHUNK - 1))
                    if o_exit == 'copy_act':
                        nc.scalar.copy(O_sb[:, s, :], O_ps[:])
                    elif o_exit == 'copy_dve':
                        nc.vector.tensor_copy(O_sb[:, s, :], O_ps[:])
                    else:
                        nc.vector.tensor_add(O_sb[:, s, :], O_ps[:], bo_sb[:])
                oq = nc.scalar if odma == 'act' else nc.sync
                for o0 in range(0, BLK, BLK // out_split):
                    o1 = o0 + BLK // out_split
                    oq.dma_start(
                        out_d[:, b * BLK + o0:b * BLK + o1, :],
                        O_sb[:, o0:o1, :])

            def back_st2(b):
                # S^T-direct: compute S^T (lhsT/rhs swapped), so exp(S^T) IS
                # the unnormalized A^T — no A-normalize multiply, no PE
                # transposes of A, no A^T psum exits. Softmax sums come from
                # all-ones matmuls (column sums broadcast to every partition
                # in PSUM); normalization is folded into the OT psum exit.
                QT, KT, V = state.pop(b)
                O_sb = po.tile([128, BLK, C], F32, tag="o")
                for s in range(BLK):
                    AT = pa.tile([128, G, 128], AMID, tag="at")
                    S_e = pss.tile([128, G // 2, 128], F32, tag="s")
                    S_o = pss.tile([128, G // 2, 128], F32, tag="s")
                    for g in range(G):
                        p0 = 64 * (g % 2)
                        S_ps = S_e if g % 2 == 0 else S_o
                        # lhsT=K^T, rhs=Q^T  ->  S^T[j, i]
                        nc.tensor.matmul(
                            S_ps[:, g // 2, :],
                            lhsT=KT[p0:p0 + 64, g // 2, s, :],
                            rhs=QT[p0:p0 + 64, g // 2, s, :],
                            start=True, stop=True)
                    # exp: AT[:, g, :] = exp(S^T_g / sqrt(gp)); even heads
                    # from S_e, odd from S_o (two batched ACT instructions).
                    nc.scalar.activation(AT[:, 0:G:2, :], S_e[:], EXP,
                                         scale=1.0 / np.sqrt(GP))
                    nc.scalar.activation(AT[:, 1:G:2, :], S_o[:], EXP,
                                         scale=1.0 / np.sqrt(GP))
                    if sums == 'allreduce':
                        # Pool cross-partition all-reduce: every partition
                        # gets all 8 heads' column sums; normalize A^T on
                        # Pool (the partition-replicated sums operand is
                        # affine) either via DVE reciprocal + Pool multiply
                        # or a direct Pool divide.
                        sums_bc = pstat.tile([128, G, 128], F32, tag="sbc")
                        nc.gpsimd.partition_all_reduce(
                            sums_bc[:], AT[:], 128, bass_isa.ReduceOp.add)
                        ATN = pa.tile([128, G, 128], AMID, tag="atn")
                        if norm == 'div':
                            nc.gpsimd.tensor_tensor(
                                ATN[:], AT[:], sums_bc[:],
                                op=mybir.AluOpType.divide)
                        else:
                            rcp_f = pstat.tile([128, G, 128], F32, tag="rcpf")
                            nc.vector.reciprocal(rcp_f[:], sums_bc[:])
                            nc.gpsimd.tensor_mul(ATN[:], AT[:], rcp_f[:])
                        OT_ps = psb.tile([128, NCHUNK, 128], F32, tag="b")
                        for g in range(G):
                            p0 = 64 * (g % 2)
                            nc.tensor.matmul(
                                OT_ps[p0:p0 + 64, g // 2, :],
                                lhsT=V[:, s, 64 * g:64 * (g + 1)],
                                rhs=ATN[:, g, :],
                                start=True, stop=True)
                        OT_sb = pot.tile([128, NCHUNK, 128], F32R, tag="ot")
                        exit_copy('ot', OT_sb[:], OT_ps[:])
                    else:
                        # column sums broadcast to all partitions:
                        # bc_e[p, q*128+i] = sum_j AT[j, 2q, i]   (even)
                        # bc_o[p, q*128+i] = sum_j AT[j, 2q+1, i] (odd)
                        bc_e = psb.tile([128, NCHUNK, 128], F32, tag="b")
                        bc_o = psb.tile([128, NCHUNK, 128], F32, tag="b")
                        nc.tensor.matmul(bc_e[:], lhsT=ones_bf[:],
                                         rhs=AT[:, 0:G:2, :], start=True,
                                         stop=True)
                        nc.tensor.matmul(bc_o[:], lhsT=ones_bf[:],
                                         rhs=AT[:, 1:G:2, :], start=True,
                                         stop=True)
                        # OT_ps[64*(g%2)+c, g//2, i] has head g=2q+(p>=64):
                        # top partitions read even-head sums, bottom odd.
                        rcp_t = pstat.tile([128, NCHUNK, 128], F32,
                                           tag="rcpt")
                        nc.vector.reciprocal(rcp_t[0:64, :, :],
                                             bc_e[0:64, :, :])
                        nc.vector.reciprocal(rcp_t[64:128, :, :],
                                             bc_o[64:128, :, :])
                        OT_ps = psb.tile([128, NCHUNK, 128], F32, tag="b")
                        for g in range(G):
                            p0 = 64 * (g % 2)
                            nc.tensor.matmul(
                                OT_ps[p0:p0 + 64, g // 2, :],
                                lhsT=V[:, s, 64 * g:64 * (g + 1)],
                                rhs=AT[:, g, :],
                                start=True, stop=True)
                        OT_sb = pot.tile([128, NCHUNK, 128], F32R, tag="ot")
                        nc.vector.tensor_mul(OT_sb[:], OT_ps[:], rcp_t[:])
                    O_ps = psb.tile([128, C], F32, tag="b")
                    for jc in range(NCHUNK):
                        nc.tensor.matmul(
                            O_ps[:], lhsT=OT_sb[:, jc, :],
                            rhs=wo_sb[:, jc, :],
                            start=(jc == 0), stop=(jc == NCHUNK - 1))
                    if o_exit == 'copy_act':
                        nc.scalar.copy(O_sb[:, s, :], O_ps[:])
                    elif o_exit == 'copy_dve':
                        nc.vector.tensor_copy(O_sb[:, s, :], O_ps[:])
                    else:
                        nc.vector.tensor_add(O_sb[:, s, :], O_ps[:], bo_sb[:])
                oq = nc.scalar if odma == 'act' else nc.sync
                for o0 in range(0, BLK, BLK // out_split):
                    o1 = o0 + BLK // out_split
                    oq.dma_start(
                        out_d[:, b * BLK + o0:b * BLK + o1, :],
                        O_sb[:, o0:o1, :])

            back_fn = back_st2 if attn_mode == 'st2' else back

            def whole():
                for b in range(nblk + 1):
                    if b < nblk:
                        front(b)
                    if b >= 1:
                        back_fn(b - 1)

            if reps == 1:
                whole()
            else:
                with tc.For_i(0, reps, 1):
                    whole()

    nc.compile()
    return nc


def make_in_maps(x, Wq, Wk, Wv, Wo, bo, num_cores=8, x_bf16=False):
    """Full inputs -> per-core input dicts (data-parallel over N)."""
    x = np.asarray(x, dtype=np.float32)
    wdt = np.float32
    if x_bf16:
        import ml_dtypes
        x = x.astype(ml_dtypes.bfloat16)
        wdt = ml_dtypes.bfloat16
    ident = np.eye(128, dtype=np.float32)
    wqT = np.ascontiguousarray(np.asarray(Wq, np.float32).T.astype(wdt))
    wkT = np.ascontiguousarray(np.asarray(Wk, np.float32).T.astype(wdt))
    wvT = np.ascontiguousarray(np.asarray(Wv, np.float32).T.astype(wdt))
    woT = np.ascontiguousarray(np.asarray(Wo, np.float32).T)
    bo = np.asarray(bo, np.float32)
    return [{"x": np.ascontiguousarray(x[n]), "wqT": wqT, "wkT": wkT,
             "wvT": wvT, "woT": woT, "bo": bo, "ident": ident}
            for n in range(num_cores)]


# Best validated config: S^T-direct attention, X^T via xbar DMA transposes
# isolated on the SP queue (all other DMA on the ACT queue — mixing xbar and
# normal descriptors on one queue corrupts), softmax sums via all-ones
# matmuls, engine-balanced PSUM exits. Pool/GpSimd deliberately unused
# (its ops are far slower on HW than the cost model suggests).
BEST_CONFIG = dict(
    attn_mode='st2', xt_mode='dma', sums='bc', odma='act', wdma='act',
    psum_bufs=(2, 3, 3), exits={'v': 'act', 'q': 'act'}, out_split=4,
    sbufs={"px": 3, "pxt": 3, "pqt": 3, "pv": 3, "pa": 4,
           "pstat": 3, "pot": 3, "po": 2})
BEST_X_BF16 = True

_NC_CACHE = {}


def kernel(x, Wq, Wk, Wv, Wo, bo):
    import numpy as np
    from concourse import bass_utils

    if "nc" not in _NC_CACHE:
        _NC_CACHE["nc"] = build_kernel(num_cores=8, **BEST_CONFIG)
    nc = _NC_CACHE["nc"]
    in_maps = make_in_maps(x, Wq, Wk, Wv, Wo, bo, num_cores=8,
                           x_bf16=BEST_X_BF16)
    res = bass_utils.run_bass_kernel_spmd(nc, in_maps, core_ids=list(range(8)))
    out = np.stack([res.results[c]["out"] for c in range(8)], axis=0)
    return out.astype(np.float32)
